# revision 1
# baseline (speedup 1.0000x reference)
"""DeepONet forward + JVPs on 8 Trainium2 NeuronCores (Bass/Tile).

Math (see reference):
  branch MLP (4x tanh layers, in_dim 1)  -> basis_br, dbr_mu   [b, 128]
  trunk  MLP (4x tanh layers, in_dim 2)  -> basis_tr, dtr_x, dtr_y  [e, 128]
  A  = basis_br * w_lin.T ; Ap = dbr_mu * w_lin.T
  U     = A  @ basis_tr.T      dU_x = A @ dtr_x.T
  dU_y  = A  @ dtr_y.T         dU_mu = Ap @ basis_tr.T
  outputs each [b, e, 1] f32

Sharding: 2x4 grid over (b, e): each core computes a [2048, 1024] block of
all four outputs.  MLPs are computed on-device in transposed layout
(features on the 128 partitions, samples on the free dim).
"""
import sys
if "/opt/trn_rl_repo" not in sys.path:
    sys.path.insert(0, "/opt/trn_rl_repo")

import numpy as np

import concourse.bass as bass
import concourse.mybir as mybir
import concourse.tile as tile
from concourse import bacc
from concourse.bass_utils import run_bass_kernel_spmd

P = 128
B_FULL, E_FULL = 4096, 4096
RB, RE = 2, 4                   # core grid: b split RB ways, e split RE ways
NB, NE = B_FULL // RB, E_FULL // RE   # per-core block: 2048 x 1024
FT = 512                        # free-dim tile (psum bank = 512 f32)
N_LAYERS = 4
F32 = mybir.dt.float32
F32R = mybir.dt.float32r
ACTF = mybir.ActivationFunctionType
ALU = mybir.AluOpType

_CACHE = {}


def _build(main_dt="f32"):
    """Build + compile the per-core kernel (same program on all 8 cores)."""
    nc = bacc.Bacc("TRN2", target_bir_lowering=False, debug=False)

    # ---- DRAM I/O ----------------------------------------------------------
    dt_t = nc.dram_tensor("dt_t", [1, NB], F32, kind="ExternalInput")
    coords_t = nc.dram_tensor("coords_t", [2, NE], F32, kind="ExternalInput")
    bW = [nc.dram_tensor("bW1", [1, P], F32, kind="ExternalInput")] + [
        nc.dram_tensor(f"bW{i+1}", [P, P], F32, kind="ExternalInput")
        for i in range(1, N_LAYERS)
    ]
    tW = [nc.dram_tensor("tW1", [2, P], F32, kind="ExternalInput")] + [
        nc.dram_tensor(f"tW{i+1}", [P, P], F32, kind="ExternalInput")
        for i in range(1, N_LAYERS)
    ]
    bB = [nc.dram_tensor(f"bB{i+1}", [P, 1], F32, kind="ExternalInput")
          for i in range(N_LAYERS)]
    tB = [nc.dram_tensor(f"tB{i+1}", [P, 1], F32, kind="ExternalInput")
          for i in range(N_LAYERS)]
    bW1c = nc.dram_tensor("bW1c", [P, 1], F32, kind="ExternalInput")   # bW1.T
    tW1c = nc.dram_tensor("tW1c", [P, 2], F32, kind="ExternalInput")   # tW1.T
    wlin = nc.dram_tensor("wlin", [P, 1], F32, kind="ExternalInput")
    outs = {
        name: nc.dram_tensor(name, [NB, NE], F32, kind="ExternalOutput")
        for name in ("U", "DX", "DY", "DMU")
    }

    mm_dt = F32 if main_dt == "f32" else F32R

    with tile.TileContext(nc) as tc:
        with (
            tc.tile_pool(name="const", bufs=1) as const,
            tc.tile_pool(name="basis", bufs=1) as basis,
            tc.tile_pool(name="work", bufs=4) as work,
            tc.tile_pool(name="stage", bufs=8) as stage,
            tc.tile_pool(name="ps", bufs=8, space="PSUM") as psp,
        ):
            # ---- load weights / inputs to SBUF -----------------------------
            def load(name, src, shape):
                t = const.tile(shape, F32, name=name, tag=name)
                nc.sync.dma_start(t[:], src.ap())
                return t

            dt_sb = load("dt_sb", dt_t, [1, NB])
            co_sb = load("co_sb", coords_t, [2, NE])
            bW_sb = [load("bW1_sb", bW[0], [1, P])] + [
                load(f"bW{i+1}_sb", bW[i], [P, P]) for i in range(1, N_LAYERS)]
            tW_sb = [load("tW1_sb", tW[0], [2, P])] + [
                load(f"tW{i+1}_sb", tW[i], [P, P]) for i in range(1, N_LAYERS)]
            bB_sb = [load(f"bB{i+1}_sb", bB[i], [P, 1]) for i in range(N_LAYERS)]
            tB_sb = [load(f"tB{i+1}_sb", tB[i], [P, 1]) for i in range(N_LAYERS)]
            bW1c_sb = load("bW1c_sb", bW1c, [P, 1])
            tW1c_sb = load("tW1c_sb", tW1c, [P, 2])
            wlin_sb = load("wlin_sb", wlin, [P, 1])

            # ---- persistent basis buffers (transposed layout) --------------
            A = basis.tile([P, NB], mm_dt, name="A", tag="A")
            Ap = basis.tile([P, NB], mm_dt, name="Ap", tag="Ap")
            Tb = basis.tile([P, NE], mm_dt, name="Tb", tag="Tb")
            Tx = basis.tile([P, NE], mm_dt, name="Tx", tag="Tx")
            Ty = basis.tile([P, NE], mm_dt, name="Ty", tag="Ty")

            # ---- one MLP+JVP column-tile ------------------------------------
            def mlp_tile(x0, col0, n_tangents, Ws, Bs, w1cols, finals):
                """Compute one [P, FT] column tile of the MLP forward +
                tangent(s) and write final layer results into `finals`
                (list of (dest_tile, scale_or_None)).  x0: [kin, *] input
                slice on partitions 0..kin-1."""
                kin = x0.shape[0]
                y = None
                ts = [None] * n_tangents
                for li in range(N_LAYERS):
                    W = Ws[li]
                    # forward pre-activation
                    pre = psp.tile([P, FT], F32, name=f"pre{col0}_{li}", tag="ps")
                    if li == 0:
                        nc.tensor.matmul(pre[:], W[:], x0[:, col0:col0 + FT],
                                         start=True, stop=True)
                    else:
                        nc.tensor.matmul(pre[:], W[:], y[:], start=True, stop=True)
                    # tangent pre-products (layers >= 1)
                    tps = []
                    if li > 0:
                        for k in range(n_tangents):
                            tp = psp.tile([P, FT], F32,
                                          name=f"tp{col0}_{li}_{k}", tag="ps")
                            nc.tensor.matmul(tp[:], W[:], ts[k][:],
                                             start=True, stop=True)
                            tps.append(tp)
                    # y = tanh(pre + b)
                    y_new = work.tile([P, FT], F32, name=f"y{col0}_{li}", tag="y")
                    nc.scalar.activation(y_new[:], pre[:], ACTF.Tanh, bias=Bs[li][:])
                    # d = 1 - y^2
                    sq = work.tile([P, FT], F32, name=f"sq{col0}_{li}", tag="sq")
                    nc.scalar.activation(sq[:], y_new[:], ACTF.Square)
                    d = work.tile([P, FT], F32, name=f"d{col0}_{li}", tag="d")
                    nc.vector.tensor_scalar(d[:], sq[:], -1.0, 1.0,
                                            ALU.mult, ALU.add)
                    # tangent updates
                    is_last = li == N_LAYERS - 1
                    t_new = []
                    for k in range(n_tangents):
                        if is_last:
                            dst, scale = finals[1 + k]
                            tk = dst[:, col0:col0 + FT]
                        else:
                            tk_t = work.tile([P, FT], F32,
                                             name=f"t{col0}_{li}_{k}", tag=f"t{k}")
                            tk = tk_t[:]
                        if li == 0:
                            # tangent seed: column k of w1cols, times d
                            nc.vector.tensor_scalar(tk, d[:],
                                                    w1cols[:, k:k + 1], None,
                                                    ALU.mult)
                        else:
                            nc.vector.tensor_mul(tk, tps[k][:], d[:])
                            if is_last and finals[1 + k][1] is not None:
                                nc.vector.tensor_scalar(tk, tk,
                                                        finals[1 + k][1][:], None,
                                                        ALU.mult)
                        if not is_last:
                            t_new.append(tk_t)
                    ts = t_new
                    if is_last:
                        dst, scale = finals[0]
                        if scale is not None:
                            nc.vector.tensor_scalar(dst[:, col0:col0 + FT],
                                                    y_new[:], scale[:], None,
                                                    ALU.mult)
                        else:
                            nc.vector.tensor_copy(dst[:, col0:col0 + FT], y_new[:])
                    y = y_new

            # trunk: basis_tr -> Tb (no scale), dtr_x -> Tx, dtr_y -> Ty
            for c in range(0, NE, FT):
                mlp_tile(co_sb, c, 2, tW_sb, tB_sb, tW1c_sb,
                         [(Tb, None), (Tx, None), (Ty, None)])
            # branch: A = y4 * wlin, Ap = t4 * wlin
            for c in range(0, NB, FT):
                mlp_tile(dt_sb, c, 1, bW_sb, bB_sb, bW1c_sb,
                         [(A, wlin_sb), (Ap, wlin_sb)])

            # ---- main loop: 4 outputs, [128 x 512] psum tiles ---------------
            rhs_for = {"U": Tb, "DX": Tx, "DY": Ty, "DMU": Tb}
            lhs_for = {"U": A, "DX": A, "DY": A, "DMU": Ap}
            order = ["U", "DX", "DY", "DMU"]
            n_nt = NE // FT
            evict_flip = 0
            for bt in range(NB // P):
                bsl = slice(bt * P, (bt + 1) * P)
                for o in order:
                    lhsT = lhs_for[o]
                    rhs = rhs_for[o]
                    st = stage.tile([P, NE], F32, name=f"st_{o}_{bt}", tag="st")
                    for nt in range(n_nt):
                        ps_t = psp.tile([P, FT], F32,
                                        name=f"mm_{o}_{bt}_{nt}", tag="ps")
                        nc.tensor.matmul(ps_t[:], lhsT[:, bsl],
                                         rhs[:, nt * FT:(nt + 1) * FT],
                                         start=True, stop=True)
                        dstv = st[:, nt * FT:(nt + 1) * FT]
                        if evict_flip % 2 == 0:
                            nc.scalar.copy(dstv, ps_t[:])
                        else:
                            nc.vector.tensor_copy(dstv, ps_t[:])
                        evict_flip += 1
                    nc.sync.dma_start(outs[o].ap()[bsl, :], st[:])

    nc.compile()
    return nc


def _get_nc(main_dt):
    if main_dt not in _CACHE:
        _CACHE[main_dt] = _build(main_dt)
    return _CACHE[main_dt]


def kernel(DT, coords, branch_Ws, branch_bs, trunk_Ws, trunk_bs, w_lin,
           _main_dt="f32", _want_results=False):
    DT = np.asarray(DT, np.float32)
    coords = np.asarray(coords, np.float32)
    nc = _get_nc(_main_dt)

    shared = {}
    for i in range(N_LAYERS):
        shared[f"bW{i+1}"] = np.ascontiguousarray(np.asarray(branch_Ws[i], np.float32))
        shared[f"tW{i+1}"] = np.ascontiguousarray(np.asarray(trunk_Ws[i], np.float32))
        shared[f"bB{i+1}"] = np.asarray(branch_bs[i], np.float32).reshape(P, 1)
        shared[f"tB{i+1}"] = np.asarray(trunk_bs[i], np.float32).reshape(P, 1)
    shared["bW1c"] = np.ascontiguousarray(shared["bW1"].T)          # [128,1]
    shared["tW1c"] = np.ascontiguousarray(shared["tW1"].T)          # [128,2]
    shared["wlin"] = np.asarray(w_lin, np.float32).reshape(P, 1)

    in_maps = []
    for c in range(RB * RE):
        rb, re = c // RE, c % RE
        m = dict(shared)
        m["dt_t"] = np.ascontiguousarray(DT[rb * NB:(rb + 1) * NB, :].T)
        m["coords_t"] = np.ascontiguousarray(coords[re * NE:(re + 1) * NE, :].T)
        in_maps.append(m)

    res = run_bass_kernel_spmd(nc, in_maps, core_ids=list(range(RB * RE)))

    full = {k: np.empty((B_FULL, E_FULL), np.float32)
            for k in ("U", "DX", "DY", "DMU")}
    for c in range(RB * RE):
        rb, re = c // RE, c % RE
        for k in full:
            full[k][rb * NB:(rb + 1) * NB, re * NE:(re + 1) * NE] = \
                res.results[c][k]
    out = tuple(full[k].reshape(B_FULL, E_FULL, 1)
                for k in ("U", "DX", "DY", "DMU"))
    if _want_results:
        return out, res
    return out
